# revision 32
# baseline (speedup 1.0000x reference)
"""Epipolar attention kernel for Trainium2 (8 NeuronCores, batch-parallel).

Math notes (derived from the reference):
  - f_tar is dead code: the output only depends on f_src / K1 / K2 / R / t.
  - With x0=0, x1=W the distance field factorizes rank-3:
        d[b,i,j] = |px_i*alpha[b,j] + py_i*beta[b,j] + gamma[b,j]|
    so 5*d = |P^T (5Q)| with P = [px;py;1] (exact in bf16).
  - softmax_j(5*(d-0.1)) == softmax_j(5*d)           (shift invariance)
  - softmax_i(1 - p)     == softmax_i(-p), p in (0,1] so exp(-p) needs no max.

Implementation notes (v5):
  - S5 = P^T (5Q) via K=6 matmuls: hi/lo bf16 split of 5Q packed into the
    contraction dim -> fp32-grade S5 in ONE matmul pass per 512 columns.
  - z = |S5|: DVE sign-mask (uint32) or ACT Abs (fp16 out), split to balance;
    row max on DVE (negated -> exp bias); e1 = exp(z-m) fp8 with fp32 s1.
  - transpose via DoubleRow fp8 matmuls against block-diagonal pair matrices
    diag(64/s1_t0, 64/s1_t1); the diag blocks are built by the DMA compute
    engine (copy 64*I once, then multiply-accumulate a broadcast 1/s1).
  - e2 = exp(-tp/64) bf16 with accumulated column sums s2; f rows scaled by
    1/s2 via DMA multiply (broadcast r2); GEMM in bf16; bf16 eviction.
  - PE p-state care: warm-up matmuls at t=0; GEMM groups are woven between
    transpose bursts so the tensor clock stays at max speed.
"""

import numpy as np
import ml_dtypes

import concourse.bass as bass
import concourse.bacc as bacc
import concourse.tile as tile
import concourse.mybir as mybir
from concourse.bass_utils import run_bass_kernel_spmd

B, C, H, W = 16, 512, 32, 32
HW = H * W          # 1024
NCORES = 8
BPC = B // NCORES   # batches per core
NT = HW // 128      # 128-row tiles per HW dim
F32 = mybir.dt.float32
F16 = mybir.dt.float16
U32 = mybir.dt.uint32
BF16 = mybir.dt.bfloat16
F8 = mybir.dt.float8e4
AF = mybir.ActivationFunctionType
AX = mybir.AxisListType
ALU = mybir.AluOpType
PM = mybir.MatmulPerfMode

N_WARM = 7           # warm-up matmuls at t=0 (PE p-state ramp)


# ---------------------------------------------------------------- host math
def _line_coeffs(K1, K2, R, t):
    """Float32 numpy mirror of the reference's per-batch line geometry.

    Returns Q (B, 3, HW) with rows [alpha, beta, gamma] and P (3, HW) with
    rows [px, py, 1].
    """
    K1 = np.asarray(K1, np.float32)
    K2 = np.asarray(K2, np.float32)
    R = np.asarray(R, np.float32)
    t = np.asarray(t, np.float32)

    z = np.zeros_like(t[:, 0])
    tx, ty, tz = t[:, 0], t[:, 1], t[:, 2]
    skew = np.stack(
        [
            np.stack([z, -tz, ty], axis=-1),
            np.stack([tz, z, -tx], axis=-1),
            np.stack([-ty, tx, z], axis=-1),
        ],
        axis=1,
    )
    E = skew @ R
    U, S, Vt = np.linalg.svd(E)
    S = S * np.array([1.0, 1.0, 0.0], dtype=S.dtype)
    E = U @ (S[:, :, None] * Vt)
    Fm = np.linalg.inv(np.swapaxes(K2, 1, 2)) @ E @ np.linalg.inv(K1)
    Fm = Fm.astype(np.float32)

    ix, iy = np.meshgrid(
        np.arange(H, dtype=np.float32), np.arange(W, dtype=np.float32), indexing="ij"
    )
    px = ix.reshape(-1)
    py = iy.reshape(-1)
    idx = np.stack([px, py, np.ones_like(px)], axis=0)  # (3, HW)

    lines = Fm @ idx[None]  # (B, 3, HW)
    a, b, c = lines[:, 0], lines[:, 1], lines[:, 2]
    x0 = np.zeros_like(a)
    y0 = -c / b
    x1 = np.full_like(a, float(W))
    y1 = -(c + a * float(W)) / b
    dx = x0 - x1
    dy = y0 - y1
    L = np.sqrt(dx * dx + dy * dy)

    alpha = dy / L
    beta = -dx / L
    gamma = (y0 * dx) / L
    Q = np.stack([alpha, beta, gamma], axis=1).astype(np.float32)  # (B, 3, HW)
    P = idx.astype(np.float32)
    return Q, P


# ---------------------------------------------------------------- device IR
def _build_nc():
    nc = bacc.Bacc("TRN2", target_bir_lowering=False, debug=False)

    p6_d = nc.dram_tensor("p6", [6, HW + 64], BF16, kind="ExternalInput")
    q6_d = nc.dram_tensor("q6", [BPC, 6, HW], BF16, kind="ExternalInput")
    fsrc_d = nc.dram_tensor("fsrc", [BPC, HW, C], BF16, kind="ExternalInput")
    ident_d = nc.dram_tensor("ident64", [128, 128], BF16, kind="ExternalInput")
    out_d = nc.dram_tensor("out", [BPC, HW, C], BF16, kind="ExternalOutput")

    with tile.TileContext(nc) as tc:
        with (
            tc.tile_pool(name="const", bufs=1) as const,
            tc.tile_pool(name="q", bufs=2) as qpool,
            tc.tile_pool(name="f", bufs=2) as fpool,
            tc.tile_pool(name="z", bufs=12) as zpool,
            tc.tile_pool(name="e", bufs=2) as epool,
            tc.tile_pool(name="e2", bufs=2) as e2pool,
            tc.tile_pool(name="stat", bufs=2) as stat,
            tc.tile_pool(name="o", bufs=4) as opool,
            tc.tile_pool(name="sps", bufs=2, space="PSUM") as spspool,
            tc.tile_pool(name="ps", bufs=2, space="PSUM") as pspool,
        ):
            # --- warm-up (no DMA dependency): ramp the PE clock -----------
            warm = const.tile([128, 512], BF16)
            nc.vector.memset(warm[:], 0.0)
            wp = spspool.tile([128, HW], F32, tag="sp")
            for k in range(N_WARM):
                nc.tensor.matmul(
                    wp[:, (k % 2) * 512 : (k % 2) * 512 + 512],
                    warm[:, 0:128],
                    warm[:],
                    start=True,
                    stop=True,
                )

            p6 = const.tile([6, HW + 64], BF16)
            nc.sync.dma_start(p6[:], p6_d[:])
            idn64 = const.tile([128, 128], BF16)
            nc.sync.dma_start(idn64[:], ident_d[:])
            # per-partition sign mask for the bitwise |S| (DVE has no abs op)
            amask = const.tile([128, 1], U32)
            nc.vector.memset(amask[:], 0x7FFFFFFF)
            # block-diag pair matrices for the DoubleRow transposes, built by
            # the DMA compute engine: zero once, copy 64*I into the two diag
            # blocks, then multiply by broadcast 1/s1 when it is ready.
            dgall = const.tile([128, 2, 4 * BPC, 256], F8)
            nc.gpsimd.memset(dgall[:], 0.0)

            st = [dict() for _ in range(BPC)]

            def load(b):
                s = st[b]
                s["q"] = qpool.tile([6, HW], BF16, tag="q", name="q")
                nc.sync.dma_start(s["q"][:], q6_d[b])
                s["fa"] = fpool.tile([128, NT, C], BF16, tag="fa", name="fa")
                for tj in range(NT):
                    nc.sync.dma_start(
                        s["fa"][:, tj, :], fsrc_d[b, tj * 128 : (tj + 1) * 128, :]
                    )
                s["ea"] = epool.tile([128, NT, HW], F8, tag="ea", name="ea")
                s["ms"] = stat.tile([128, NT], F32, tag="ms", name="ms")
                s["s1"] = stat.tile([128, NT], F32, tag="s1", name="s1")
                s["r1"] = stat.tile([128, NT], F32, tag="r1", name="r1")
                s["e2"] = e2pool.tile([128, NT, HW], BF16, tag="e2", name="e2")
                s["s2"] = stat.tile([128, NT], F32, tag="s2", name="s2")
                s["r2"] = stat.tile([128, NT], F32, tag="r2", name="r2")

            def s_matmul(b, ti):
                s = st[b]
                sp = spspool.tile([128, HW], F32, tag="sp")
                for nh in range(2):
                    nc.tensor.matmul(
                        sp[:, nh * 512 : (nh + 1) * 512],
                        p6[:, ti * 128 : (ti + 1) * 128],
                        s["q"][:, nh * 512 : (nh + 1) * 512],
                        start=True,
                        stop=True,
                    )
                return sp

            def z_max(b, ti, sp, eng):
                # z = |S5|: DVE path via uint32 sign-mask (fp32 z), ACT path
                # via Abs (fp16 z: z <= ~300 here and entries far below the
                # row max don't matter). Row max (negated) on DVE.
                s = st[b]
                if eng == "dve":
                    zt = zpool.tile([128, HW], F32, tag="z32")
                    nc.vector.tensor_scalar(
                        zt[:].bitcast(U32),
                        sp[:].bitcast(U32),
                        amask[:, 0:1],
                        None,
                        op0=ALU.bitwise_and,
                    )
                else:
                    zt = zpool.tile([128, HW], F16, tag="z16")
                    nc.scalar.activation(zt[:], sp[:], AF.Abs)
                nc.vector.reduce_max(
                    s["ms"][:, ti : ti + 1], zt[:], axis=AX.X, negate=True
                )
                return zt

            def e1_pass(b, ti, zt):
                # e1 = exp(z - m) fp8 with fp32 row-sum accum; on odd ti also
                # r1 = 1/s1 (DVE) and the two diag blocks scaled by r1 (DMA).
                s = st[b]
                nc.scalar.activation(
                    s["ea"][:, ti, :],
                    zt[:],
                    AF.Exp,
                    bias=s["ms"][:, ti : ti + 1],
                    accum_out=s["s1"][:, ti : ti + 1],
                )
                if ti % 2 == 1:
                    g = ti // 2
                    nc.vector.reciprocal(
                        s["r1"][:, ti - 1 : ti + 1], s["s1"][:, ti - 1 : ti + 1]
                    )
                    for m in range(2):
                        nc.vector.tensor_scalar_mul(
                            dgall[:, m, g + 4 * b, m * 128 : (m + 1) * 128],
                            idn64[:],
                            s["r1"][:, ti - 1 + m : ti + m],
                        )

            def t_mms(b, tj):
                # tp[j, i] = 64 * p^T via DoubleRow fp8 matmuls against the
                # block-diag pair matrices.
                s = st[b]
                tp = pspool.tile([128, HW], F32, tag="ps")
                for g in range(4):
                    nc.tensor.matmul(
                        tp[:, g * 256 : (g + 1) * 256],
                        s["ea"][:, 2 * g : 2 * g + 2, tj * 128 : (tj + 1) * 128],
                        dgall[:, :, g + 4 * b, :],
                        start=True,
                        stop=True,
                        perf_mode=PM.DoubleRow,
                    )
                return tp

            def e2_pass(b, tj, tp):
                # e2 = exp(-tp/64) with s2 accum; r2 = 1/s2 (DVE); fold r2
                # into the f rows via DMA multiply (broadcast r2).
                s = st[b]
                nc.scalar.activation(
                    s["e2"][:, tj, :],
                    tp[:],
                    AF.Exp,
                    scale=-1.0 / 64.0,
                    accum_out=s["s2"][:, tj : tj + 1],
                )
                nc.vector.reciprocal(
                    s["r2"][:, tj : tj + 1], s["s2"][:, tj : tj + 1]
                )
                nc.vector.tensor_scalar_mul(
                    s["fa"][:, tj, :], s["fa"][:, tj, :], s["r2"][:, tj : tj + 1]
                )

            def gemm_group(b, g):
                # GEMM: out[i, c] = sum_j e2[j, i] * fw[j, c]; two i-tiles per
                # 2-bank PSUM slot.
                s = st[b]
                og = ogpool.tile([128, 2, C], F32, tag="og")
                for tj in range(NT):
                    for half in range(2):
                        oi = 2 * g + half
                        nc.tensor.matmul(
                            og[:, half, :],
                            s["e2"][:, tj, oi * 128 : (oi + 1) * 128],
                            s["fa"][:, tj, :],
                            start=(tj == 0),
                            stop=(tj == NT - 1),
                        )
                return og

            def evict(b, g, og, eng):
                ob = opool.tile([128, 2, C], BF16)
                if eng == 0:
                    nc.scalar.copy(ob[:], og[:])
                else:
                    nc.vector.tensor_copy(ob[:], og[:])
                nc.sync.dma_start(
                    out_d[b, g * 256 : (g + 1) * 256, :].rearrange(
                        "(t p) c -> p t c", p=128
                    ),
                    ob[:],
                )

            # ---- emission schedule (3-phase software pipeline) ----------
            # A: S(b0) + first softmax of b0     (EW-latency bound)
            # B: S(b1)+z(b1)+e1(b1) interleaved with T(b0)+e2(b0) and the
            #    first two GEMM groups of b0 (k-first paced by the e2 stream)
            # C: remaining GEMM groups of b0 woven with T(b1)+e2(b1), then
            #    GEMM of b1 (PE stays dense and hot throughout)
            load(0)
            load(1)
            for ti in range(NT):
                sp = s_matmul(0, ti)
                zt = z_max(0, ti, sp, eng="dve" if ti % 2 == 0 else "act")
                e1_pass(0, ti, zt)

            def g_step(b, og, g, tj, s):
                for half in range(2):
                    oi = 2 * g + half
                    nc.tensor.matmul(
                        og[:, half, :],
                        s["e2"][:, tj, oi * 128 : (oi + 1) * 128],
                        s["fa"][:, tj, :],
                        start=(tj == 0),
                        stop=(tj == NT - 1),
                    )

            # phase B: S(b1) + first softmax of b1 interleaved with
            # T(b0) + e2(b0) + famul(b0)
            for k in range(NT):
                sp = s_matmul(1, k)
                t_mms_tp = t_mms(0, k)
                e2_pass(0, k, t_mms_tp)
                zt = z_max(1, k, sp, eng="dve" if k % 2 == 0 else "act")
                e1_pass(1, k, zt)

            # phase C: GEMM(b0) groups dense with T(b1)/e2(b1) pairs woven
            # between; b0 evicts go to DVE so ACT stays exclusive to e2(b1)
            # (an ACT-queued evict would delay tp recycling and stall PE).
            b1_todo = list(range(NT))
            for g in range(4):
                og = spspool.tile([128, 2, C], F32, tag="sp")
                for tj in range(NT):
                    g_step(0, og, g, tj, st[0])
                    if tj in (2, 5) and b1_todo:
                        kk = b1_todo.pop(0)
                        tp = t_mms(1, kk)
                        e2_pass(1, kk, tp)
                evict(0, g, og, eng=g % 2)
            for kk in b1_todo:
                tp = t_mms(1, kk)
                e2_pass(1, kk, tp)
            for g in range(4):
                og = spspool.tile([128, 2, C], F32, tag="sp")
                for tj in range(NT):
                    g_step(1, og, g, tj, st[1])
                if g < 3:
                    evict(1, g, og, eng=g % 2)
                else:
                    # split the last eviction across both engines so the
                    # final DMA starts sooner
                    ob = opool.tile([128, 2, C], BF16)
                    nc.scalar.copy(ob[:, 0, :], og[:, 0, :])
                    nc.vector.tensor_copy(ob[:, 1, :], og[:, 1, :])
                    nc.sync.dma_start(
                        out_d[1, g * 256 : g * 256 + 128, :].rearrange(
                            "(t p) c -> p t c", p=128
                        ),
                        ob[:, 0:1, :],
                    )
                    nc.sync.dma_start(
                        out_d[1, g * 256 + 128 : (g + 1) * 256, :].rearrange(
                            "(t p) c -> p t c", p=128
                        ),
                        ob[:, 1:2, :],
                    )
    nc.compile()
    return nc


_NC = None


def _get_nc():
    global _NC
    if _NC is None:
        _NC = _build_nc()
    return _NC


# ---------------------------------------------------------------- execution
def _run(inputs, trace=False):
    f_src = np.asarray(inputs["f_src"], np.float32)
    Q, P = _line_coeffs(inputs["K1"], inputs["K2"], inputs["R"], inputs["t"])
    Q5 = 5.0 * Q

    fsrcT = np.ascontiguousarray(
        f_src.reshape(B, C, HW).transpose(0, 2, 1)
    ).astype(ml_dtypes.bfloat16)
    ident64 = (64.0 * np.eye(128, dtype=np.float32)).astype(ml_dtypes.bfloat16)

    q_hi = Q5.astype(ml_dtypes.bfloat16)
    q_lo = (Q5 - q_hi.astype(np.float32)).astype(ml_dtypes.bfloat16)
    q6 = np.concatenate([q_hi, q_lo], axis=1)  # (B, 6, HW) bf16
    # pixel columns plus the 64 coarse-grid nodes (8x8 cell centers)
    nx = np.arange(8, dtype=np.float32) * 4 + 2.0
    cpx, cpy = np.meshgrid(nx, nx, indexing="ij")
    Pc = np.stack(
        [cpx.reshape(-1), cpy.reshape(-1), np.ones(64, np.float32)], axis=0
    )
    Pe = np.concatenate([P, Pc], axis=1)  # (3, HW+64)
    p6 = np.concatenate([Pe, Pe], axis=0).astype(ml_dtypes.bfloat16)  # exact
    # selection matrices: sel[node, ti, p] = 1 iff node owns row 128*ti+p
    pxi = (np.arange(HW) // 32) // 4
    pyi = (np.arange(HW) % 32) // 4
    node_of = pxi * 8 + pyi
    sel = np.zeros((64, NT, 128), np.float32)
    for i in range(HW):
        sel[node_of[i], i // 128, i % 128] = 1.0
    sel = sel.astype(ml_dtypes.bfloat16)

    in_maps = []
    for core in range(NCORES):
        lo = core * BPC
        hi = lo + BPC
        in_maps.append(
            {
                "p6": p6,
                "q6": np.ascontiguousarray(q6[lo:hi]),
                "fsrc": np.ascontiguousarray(fsrcT[lo:hi]),
                "ident64": ident64,
            }
        )

    nc = _get_nc()
    res = run_bass_kernel_spmd(nc, in_maps, list(range(NCORES)), trace=trace)
    out_flat = np.concatenate(
        [res.results[i]["out"] for i in range(NCORES)], axis=0
    )  # (B, HW, C) bf16
    out = np.ascontiguousarray(out_flat).astype(np.float32).reshape(B, C, H, W)
    return out, res


def kernel(**inputs):
    out, _ = _run(inputs, trace=False)
    return out


# revision 33
# speedup vs baseline: 1.0106x; 1.0106x over previous
"""Epipolar attention kernel for Trainium2 (8 NeuronCores, batch-parallel).

Math notes (derived from the reference):
  - f_tar is dead code: the output only depends on f_src / K1 / K2 / R / t.
  - With x0=0, x1=W the distance field factorizes rank-3:
        d[b,i,j] = |px_i*alpha[b,j] + py_i*beta[b,j] + gamma[b,j]|
    so 5*d = |P^T (5Q)| with P = [px;py;1] (exact in bf16).
  - softmax_j(5*(d-0.1)) == softmax_j(5*d)           (shift invariance)
  - softmax_i(1 - p)     == softmax_i(-p), p in (0,1] so exp(-p) needs no max.

Implementation notes (v5):
  - S5 = P^T (5Q) via K=6 matmuls: hi/lo bf16 split of 5Q packed into the
    contraction dim -> fp32-grade S5 in ONE matmul pass per 512 columns.
  - z = |S5|: DVE sign-mask (uint32) or ACT Abs (fp16 out), split to balance;
    row max on DVE (negated -> exp bias); e1 = exp(z-m) fp8 with fp32 s1.
  - transpose via DoubleRow fp8 matmuls against block-diagonal pair matrices
    diag(64/s1_t0, 64/s1_t1); the diag blocks are built by the DMA compute
    engine (copy 64*I once, then multiply-accumulate a broadcast 1/s1).
  - e2 = exp(-tp/64) bf16 with accumulated column sums s2; f rows scaled by
    1/s2 via DMA multiply (broadcast r2); GEMM in bf16; bf16 eviction.
  - PE p-state care: warm-up matmuls at t=0; GEMM groups are woven between
    transpose bursts so the tensor clock stays at max speed.
"""

import numpy as np
import ml_dtypes

import concourse.bass as bass
import concourse.bacc as bacc
import concourse.tile as tile
import concourse.mybir as mybir
from concourse.bass_utils import run_bass_kernel_spmd

B, C, H, W = 16, 512, 32, 32
HW = H * W          # 1024
NCORES = 8
BPC = B // NCORES   # batches per core
NT = HW // 128      # 128-row tiles per HW dim
F32 = mybir.dt.float32
F16 = mybir.dt.float16
U32 = mybir.dt.uint32
BF16 = mybir.dt.bfloat16
F8 = mybir.dt.float8e4
AF = mybir.ActivationFunctionType
AX = mybir.AxisListType
ALU = mybir.AluOpType
PM = mybir.MatmulPerfMode

N_WARM = 7           # warm-up matmuls at t=0 (PE p-state ramp)


# ---------------------------------------------------------------- host math
def _line_coeffs(K1, K2, R, t):
    """Float32 numpy mirror of the reference's per-batch line geometry.

    Returns Q (B, 3, HW) with rows [alpha, beta, gamma] and P (3, HW) with
    rows [px, py, 1].
    """
    K1 = np.asarray(K1, np.float32)
    K2 = np.asarray(K2, np.float32)
    R = np.asarray(R, np.float32)
    t = np.asarray(t, np.float32)

    z = np.zeros_like(t[:, 0])
    tx, ty, tz = t[:, 0], t[:, 1], t[:, 2]
    skew = np.stack(
        [
            np.stack([z, -tz, ty], axis=-1),
            np.stack([tz, z, -tx], axis=-1),
            np.stack([-ty, tx, z], axis=-1),
        ],
        axis=1,
    )
    E = skew @ R
    U, S, Vt = np.linalg.svd(E)
    S = S * np.array([1.0, 1.0, 0.0], dtype=S.dtype)
    E = U @ (S[:, :, None] * Vt)
    Fm = np.linalg.inv(np.swapaxes(K2, 1, 2)) @ E @ np.linalg.inv(K1)
    Fm = Fm.astype(np.float32)

    ix, iy = np.meshgrid(
        np.arange(H, dtype=np.float32), np.arange(W, dtype=np.float32), indexing="ij"
    )
    px = ix.reshape(-1)
    py = iy.reshape(-1)
    idx = np.stack([px, py, np.ones_like(px)], axis=0)  # (3, HW)

    lines = Fm @ idx[None]  # (B, 3, HW)
    a, b, c = lines[:, 0], lines[:, 1], lines[:, 2]
    x0 = np.zeros_like(a)
    y0 = -c / b
    x1 = np.full_like(a, float(W))
    y1 = -(c + a * float(W)) / b
    dx = x0 - x1
    dy = y0 - y1
    L = np.sqrt(dx * dx + dy * dy)

    alpha = dy / L
    beta = -dx / L
    gamma = (y0 * dx) / L
    Q = np.stack([alpha, beta, gamma], axis=1).astype(np.float32)  # (B, 3, HW)
    P = idx.astype(np.float32)
    return Q, P


# ---------------------------------------------------------------- device IR
def _build_nc():
    nc = bacc.Bacc("TRN2", target_bir_lowering=False, debug=False)

    p6_d = nc.dram_tensor("p6", [6, HW + 64], BF16, kind="ExternalInput")
    q6_d = nc.dram_tensor("q6", [BPC, 6, HW], BF16, kind="ExternalInput")
    fsrc_d = nc.dram_tensor("fsrc", [BPC, HW, C], BF16, kind="ExternalInput")
    ident_d = nc.dram_tensor("ident64", [128, 128], BF16, kind="ExternalInput")
    out_d = nc.dram_tensor("out", [BPC, HW, C], BF16, kind="ExternalOutput")

    with tile.TileContext(nc) as tc:
        with (
            tc.tile_pool(name="const", bufs=1) as const,
            tc.tile_pool(name="q", bufs=2) as qpool,
            tc.tile_pool(name="f", bufs=2) as fpool,
            tc.tile_pool(name="z", bufs=12) as zpool,
            tc.tile_pool(name="e", bufs=2) as epool,
            tc.tile_pool(name="e2", bufs=2) as e2pool,
            tc.tile_pool(name="stat", bufs=2) as stat,
            tc.tile_pool(name="o", bufs=4) as opool,
            tc.tile_pool(name="sps", bufs=2, space="PSUM") as spspool,
            tc.tile_pool(name="ps", bufs=2, space="PSUM") as pspool,
        ):
            # --- warm-up (no DMA dependency): ramp the PE clock -----------
            warm = const.tile([128, 512], BF16)
            nc.vector.memset(warm[:], 0.0)
            wp = spspool.tile([128, HW], F32, tag="sp")
            for k in range(N_WARM):
                nc.tensor.matmul(
                    wp[:, (k % 2) * 512 : (k % 2) * 512 + 512],
                    warm[:, 0:128],
                    warm[:],
                    start=True,
                    stop=True,
                )

            p6 = const.tile([6, HW + 64], BF16)
            nc.sync.dma_start(p6[:], p6_d[:])
            idn64 = const.tile([128, 128], BF16)
            nc.sync.dma_start(idn64[:], ident_d[:])
            # per-partition sign mask for the bitwise |S| (DVE has no abs op)
            amask = const.tile([128, 1], U32)
            nc.vector.memset(amask[:], 0x7FFFFFFF)
            # block-diag pair matrices for the DoubleRow transposes, built by
            # the DMA compute engine: zero once, copy 64*I into the two diag
            # blocks, then multiply by broadcast 1/s1 when it is ready.
            dgall = const.tile([128, 2, 4 * BPC, 256], F8)
            nc.gpsimd.memset(dgall[:], 0.0)

            st = [dict() for _ in range(BPC)]

            def load(b):
                s = st[b]
                s["q"] = qpool.tile([6, HW], BF16, tag="q", name="q")
                nc.sync.dma_start(s["q"][:], q6_d[b])
                s["fa"] = fpool.tile([128, NT, C], BF16, tag="fa", name="fa")
                for tj in range(NT):
                    nc.sync.dma_start(
                        s["fa"][:, tj, :], fsrc_d[b, tj * 128 : (tj + 1) * 128, :]
                    )
                s["ea"] = epool.tile([128, NT, HW], F8, tag="ea", name="ea")
                s["ms"] = stat.tile([128, NT], F32, tag="ms", name="ms")
                s["s1"] = stat.tile([128, NT], F32, tag="s1", name="s1")
                s["r1"] = stat.tile([128, NT], F32, tag="r1", name="r1")
                s["e2"] = e2pool.tile([128, NT, HW], BF16, tag="e2", name="e2")
                s["s2"] = stat.tile([128, NT], F32, tag="s2", name="s2")
                s["r2"] = stat.tile([128, NT], F32, tag="r2", name="r2")

            def s_matmul(b, ti):
                s = st[b]
                sp = spspool.tile([128, HW], F32, tag="sp")
                for nh in range(2):
                    nc.tensor.matmul(
                        sp[:, nh * 512 : (nh + 1) * 512],
                        p6[:, ti * 128 : (ti + 1) * 128],
                        s["q"][:, nh * 512 : (nh + 1) * 512],
                        start=True,
                        stop=True,
                    )
                return sp

            def z_max(b, ti, sp, eng):
                # z = |S5|: DVE path via uint32 sign-mask (fp32 z), ACT path
                # via Abs (fp16 z: z <= ~300 here and entries far below the
                # row max don't matter). Row max (negated) on DVE.
                s = st[b]
                if eng == "dve":
                    zt = zpool.tile([128, HW], F32, tag="z32")
                    nc.vector.tensor_scalar(
                        zt[:].bitcast(U32),
                        sp[:].bitcast(U32),
                        amask[:, 0:1],
                        None,
                        op0=ALU.bitwise_and,
                    )
                else:
                    zt = zpool.tile([128, HW], F16, tag="z16")
                    nc.scalar.activation(zt[:], sp[:], AF.Abs)
                nc.vector.reduce_max(
                    s["ms"][:, ti : ti + 1], zt[:], axis=AX.X, negate=True
                )
                return zt

            def e1_pass(b, ti, zt):
                # e1 = exp(z - m) fp8 with fp32 row-sum accum; on odd ti also
                # r1 = 1/s1 (DVE) and the two diag blocks scaled by r1 (DMA).
                s = st[b]
                nc.scalar.activation(
                    s["ea"][:, ti, :],
                    zt[:],
                    AF.Exp,
                    bias=s["ms"][:, ti : ti + 1],
                    accum_out=s["s1"][:, ti : ti + 1],
                )
                if ti % 2 == 1:
                    g = ti // 2
                    nc.vector.reciprocal(
                        s["r1"][:, ti - 1 : ti + 1], s["s1"][:, ti - 1 : ti + 1]
                    )
                    for m in range(2):
                        nc.vector.tensor_scalar_mul(
                            dgall[:, m, g + 4 * b, m * 128 : (m + 1) * 128],
                            idn64[:],
                            s["r1"][:, ti - 1 + m : ti + m],
                        )

            def t_mms(b, tj):
                # tp[j, i] = 64 * p^T via DoubleRow fp8 matmuls against the
                # block-diag pair matrices.
                s = st[b]
                tp = pspool.tile([128, HW], F32, tag="ps")
                for g in range(4):
                    nc.tensor.matmul(
                        tp[:, g * 256 : (g + 1) * 256],
                        s["ea"][:, 2 * g : 2 * g + 2, tj * 128 : (tj + 1) * 128],
                        dgall[:, :, g + 4 * b, :],
                        start=True,
                        stop=True,
                        perf_mode=PM.DoubleRow,
                    )
                return tp

            def e2_pass(b, tj, tp):
                # e2 = exp(-tp/64) with s2 accum; r2 = 1/s2 (DVE); fold r2
                # into the f rows via DMA multiply (broadcast r2).
                s = st[b]
                nc.scalar.activation(
                    s["e2"][:, tj, :],
                    tp[:],
                    AF.Exp,
                    scale=-1.0 / 64.0,
                    accum_out=s["s2"][:, tj : tj + 1],
                )
                nc.vector.reciprocal(
                    s["r2"][:, tj : tj + 1], s["s2"][:, tj : tj + 1]
                )
                nc.vector.tensor_scalar_mul(
                    s["fa"][:, tj, :], s["fa"][:, tj, :], s["r2"][:, tj : tj + 1]
                )

            def gemm_group(b, g):
                # GEMM: out[i, c] = sum_j e2[j, i] * fw[j, c]; two i-tiles per
                # 2-bank PSUM slot.
                s = st[b]
                og = ogpool.tile([128, 2, C], F32, tag="og")
                for tj in range(NT):
                    for half in range(2):
                        oi = 2 * g + half
                        nc.tensor.matmul(
                            og[:, half, :],
                            s["e2"][:, tj, oi * 128 : (oi + 1) * 128],
                            s["fa"][:, tj, :],
                            start=(tj == 0),
                            stop=(tj == NT - 1),
                        )
                return og

            def evict(b, g, og, eng):
                ob = opool.tile([128, 2, C], BF16)
                if eng == 0:
                    nc.scalar.copy(ob[:], og[:])
                else:
                    nc.vector.tensor_copy(ob[:], og[:])
                nc.sync.dma_start(
                    out_d[b, g * 256 : (g + 1) * 256, :].rearrange(
                        "(t p) c -> p t c", p=128
                    ),
                    ob[:],
                )

            # ---- emission schedule (3-phase software pipeline) ----------
            # A: S(b0) + first softmax of b0     (EW-latency bound)
            # B: S(b1)+z(b1)+e1(b1) interleaved with T(b0)+e2(b0) and the
            #    first two GEMM groups of b0 (k-first paced by the e2 stream)
            # C: remaining GEMM groups of b0 woven with T(b1)+e2(b1), then
            #    GEMM of b1 (PE stays dense and hot throughout)
            load(0)
            load(1)
            for ti in range(NT):
                sp = s_matmul(0, ti)
                zt = z_max(0, ti, sp, eng="dve" if ti % 2 == 0 else "act")
                e1_pass(0, ti, zt)

            def g_step(b, og, g, tj, s):
                for half in range(2):
                    oi = 2 * g + half
                    nc.tensor.matmul(
                        og[:, half, :],
                        s["e2"][:, tj, oi * 128 : (oi + 1) * 128],
                        s["fa"][:, tj, :],
                        start=(tj == 0),
                        stop=(tj == NT - 1),
                    )

            # phase B: S(b1) + first softmax of b1 interleaved with
            # T(b0) + e2(b0) + famul(b0)
            for k in range(NT):
                sp = s_matmul(1, k)
                t_mms_tp = t_mms(0, k)
                e2_pass(0, k, t_mms_tp)
                zt = z_max(1, k, sp, eng="dve" if k % 3 != 1 else "act")
                e1_pass(1, k, zt)

            # phase C: GEMM(b0) groups dense with T(b1)/e2(b1) pairs woven
            # between; b0 evicts go to DVE so ACT stays exclusive to e2(b1)
            # (an ACT-queued evict would delay tp recycling and stall PE).
            b1_todo = list(range(NT))
            for g in range(4):
                og = spspool.tile([128, 2, C], F32, tag="sp")
                for tj in range(NT):
                    g_step(0, og, g, tj, st[0])
                    if tj in (2, 5) and b1_todo:
                        kk = b1_todo.pop(0)
                        tp = t_mms(1, kk)
                        e2_pass(1, kk, tp)
                evict(0, g, og, eng=g % 2)
            for kk in b1_todo:
                tp = t_mms(1, kk)
                e2_pass(1, kk, tp)
            for g in range(4):
                og = spspool.tile([128, 2, C], F32, tag="sp")
                for tj in range(NT):
                    g_step(1, og, g, tj, st[1])
                if g < 3:
                    evict(1, g, og, eng=g % 2)
                else:
                    # split the last eviction across both engines so the
                    # final DMA starts sooner
                    ob = opool.tile([128, 2, C], BF16)
                    nc.scalar.copy(ob[:, 0, :], og[:, 0, :])
                    nc.vector.tensor_copy(ob[:, 1, :], og[:, 1, :])
                    nc.sync.dma_start(
                        out_d[1, g * 256 : g * 256 + 128, :].rearrange(
                            "(t p) c -> p t c", p=128
                        ),
                        ob[:, 0:1, :],
                    )
                    nc.sync.dma_start(
                        out_d[1, g * 256 + 128 : (g + 1) * 256, :].rearrange(
                            "(t p) c -> p t c", p=128
                        ),
                        ob[:, 1:2, :],
                    )
    nc.compile()
    return nc


_NC = None


def _get_nc():
    global _NC
    if _NC is None:
        _NC = _build_nc()
    return _NC


# ---------------------------------------------------------------- execution
def _run(inputs, trace=False):
    f_src = np.asarray(inputs["f_src"], np.float32)
    Q, P = _line_coeffs(inputs["K1"], inputs["K2"], inputs["R"], inputs["t"])
    Q5 = 5.0 * Q

    fsrcT = np.ascontiguousarray(
        f_src.reshape(B, C, HW).transpose(0, 2, 1)
    ).astype(ml_dtypes.bfloat16)
    ident64 = (64.0 * np.eye(128, dtype=np.float32)).astype(ml_dtypes.bfloat16)

    q_hi = Q5.astype(ml_dtypes.bfloat16)
    q_lo = (Q5 - q_hi.astype(np.float32)).astype(ml_dtypes.bfloat16)
    q6 = np.concatenate([q_hi, q_lo], axis=1)  # (B, 6, HW) bf16
    # pixel columns plus the 64 coarse-grid nodes (8x8 cell centers)
    nx = np.arange(8, dtype=np.float32) * 4 + 2.0
    cpx, cpy = np.meshgrid(nx, nx, indexing="ij")
    Pc = np.stack(
        [cpx.reshape(-1), cpy.reshape(-1), np.ones(64, np.float32)], axis=0
    )
    Pe = np.concatenate([P, Pc], axis=1)  # (3, HW+64)
    p6 = np.concatenate([Pe, Pe], axis=0).astype(ml_dtypes.bfloat16)  # exact
    # selection matrices: sel[node, ti, p] = 1 iff node owns row 128*ti+p
    pxi = (np.arange(HW) // 32) // 4
    pyi = (np.arange(HW) % 32) // 4
    node_of = pxi * 8 + pyi
    sel = np.zeros((64, NT, 128), np.float32)
    for i in range(HW):
        sel[node_of[i], i // 128, i % 128] = 1.0
    sel = sel.astype(ml_dtypes.bfloat16)

    in_maps = []
    for core in range(NCORES):
        lo = core * BPC
        hi = lo + BPC
        in_maps.append(
            {
                "p6": p6,
                "q6": np.ascontiguousarray(q6[lo:hi]),
                "fsrc": np.ascontiguousarray(fsrcT[lo:hi]),
                "ident64": ident64,
            }
        )

    nc = _get_nc()
    res = run_bass_kernel_spmd(nc, in_maps, list(range(NCORES)), trace=trace)
    out_flat = np.concatenate(
        [res.results[i]["out"] for i in range(NCORES)], axis=0
    )  # (B, HW, C) bf16
    out = np.ascontiguousarray(out_flat).astype(np.float32).reshape(B, C, H, W)
    return out, res


def kernel(**inputs):
    out, _ = _run(inputs, trace=False)
    return out


# revision 35
# speedup vs baseline: 1.0157x; 1.0051x over previous
"""Epipolar attention kernel for Trainium2 (8 NeuronCores, batch-parallel).

Math notes (derived from the reference):
  - f_tar is dead code: the output only depends on f_src / K1 / K2 / R / t.
  - With x0=0, x1=W the distance field factorizes rank-3:
        d[b,i,j] = |px_i*alpha[b,j] + py_i*beta[b,j] + gamma[b,j]|
    so 5*d = |P^T (5Q)| with P = [px;py;1] (exact in bf16).
  - softmax_j(5*(d-0.1)) == softmax_j(5*d)           (shift invariance)
  - softmax_i(1 - p)     == softmax_i(-p), p in (0,1] so exp(-p) needs no max.

Implementation notes (v5):
  - S5 = P^T (5Q) via K=6 matmuls: hi/lo bf16 split of 5Q packed into the
    contraction dim -> fp32-grade S5 in ONE matmul pass per 512 columns.
  - z = |S5|: DVE sign-mask (uint32) or ACT Abs (fp16 out), split to balance;
    row max on DVE (negated -> exp bias); e1 = exp(z-m) fp8 with fp32 s1.
  - transpose via DoubleRow fp8 matmuls against block-diagonal pair matrices
    diag(64/s1_t0, 64/s1_t1); the diag blocks are built by the DMA compute
    engine (copy 64*I once, then multiply-accumulate a broadcast 1/s1).
  - e2 = exp(-tp/64) bf16 with accumulated column sums s2; f rows scaled by
    1/s2 via DMA multiply (broadcast r2); GEMM in bf16; bf16 eviction.
  - PE p-state care: warm-up matmuls at t=0; GEMM groups are woven between
    transpose bursts so the tensor clock stays at max speed.
"""

import numpy as np
import ml_dtypes

import concourse.bass as bass
import concourse.bacc as bacc
import concourse.tile as tile
import concourse.mybir as mybir
from concourse.bass_utils import run_bass_kernel_spmd

B, C, H, W = 16, 512, 32, 32
HW = H * W          # 1024
NCORES = 8
BPC = B // NCORES   # batches per core
NT = HW // 128      # 128-row tiles per HW dim
F32 = mybir.dt.float32
F16 = mybir.dt.float16
U32 = mybir.dt.uint32
BF16 = mybir.dt.bfloat16
F8 = mybir.dt.float8e4
AF = mybir.ActivationFunctionType
AX = mybir.AxisListType
ALU = mybir.AluOpType
PM = mybir.MatmulPerfMode

N_WARM = 7           # warm-up matmuls at t=0 (PE p-state ramp)


# ---------------------------------------------------------------- host math
def _line_coeffs(K1, K2, R, t):
    """Float32 numpy mirror of the reference's per-batch line geometry.

    Returns Q (B, 3, HW) with rows [alpha, beta, gamma] and P (3, HW) with
    rows [px, py, 1].
    """
    K1 = np.asarray(K1, np.float32)
    K2 = np.asarray(K2, np.float32)
    R = np.asarray(R, np.float32)
    t = np.asarray(t, np.float32)

    z = np.zeros_like(t[:, 0])
    tx, ty, tz = t[:, 0], t[:, 1], t[:, 2]
    skew = np.stack(
        [
            np.stack([z, -tz, ty], axis=-1),
            np.stack([tz, z, -tx], axis=-1),
            np.stack([-ty, tx, z], axis=-1),
        ],
        axis=1,
    )
    E = skew @ R
    U, S, Vt = np.linalg.svd(E)
    S = S * np.array([1.0, 1.0, 0.0], dtype=S.dtype)
    E = U @ (S[:, :, None] * Vt)
    Fm = np.linalg.inv(np.swapaxes(K2, 1, 2)) @ E @ np.linalg.inv(K1)
    Fm = Fm.astype(np.float32)

    ix, iy = np.meshgrid(
        np.arange(H, dtype=np.float32), np.arange(W, dtype=np.float32), indexing="ij"
    )
    px = ix.reshape(-1)
    py = iy.reshape(-1)
    idx = np.stack([px, py, np.ones_like(px)], axis=0)  # (3, HW)

    lines = Fm @ idx[None]  # (B, 3, HW)
    a, b, c = lines[:, 0], lines[:, 1], lines[:, 2]
    x0 = np.zeros_like(a)
    y0 = -c / b
    x1 = np.full_like(a, float(W))
    y1 = -(c + a * float(W)) / b
    dx = x0 - x1
    dy = y0 - y1
    L = np.sqrt(dx * dx + dy * dy)

    alpha = dy / L
    beta = -dx / L
    gamma = (y0 * dx) / L
    Q = np.stack([alpha, beta, gamma], axis=1).astype(np.float32)  # (B, 3, HW)
    P = idx.astype(np.float32)
    return Q, P


# ---------------------------------------------------------------- device IR
def _build_nc():
    nc = bacc.Bacc("TRN2", target_bir_lowering=False, debug=False)

    p6_d = nc.dram_tensor("p6", [6, HW + 64], BF16, kind="ExternalInput")
    q6_d = nc.dram_tensor("q6", [BPC, 6, HW], BF16, kind="ExternalInput")
    fsrc_d = nc.dram_tensor("fsrc", [BPC, HW, C], BF16, kind="ExternalInput")
    ident_d = nc.dram_tensor("ident64", [128, 128], BF16, kind="ExternalInput")
    out_d = nc.dram_tensor("out", [BPC, HW, C], BF16, kind="ExternalOutput")

    with tile.TileContext(nc) as tc:
        with (
            tc.tile_pool(name="const", bufs=1) as const,
            tc.tile_pool(name="q", bufs=2) as qpool,
            tc.tile_pool(name="f", bufs=2) as fpool,
            tc.tile_pool(name="z", bufs=12) as zpool,
            tc.tile_pool(name="e", bufs=2) as epool,
            tc.tile_pool(name="e2", bufs=2) as e2pool,
            tc.tile_pool(name="stat", bufs=2) as stat,
            tc.tile_pool(name="o", bufs=4) as opool,
            tc.tile_pool(name="sps", bufs=2, space="PSUM") as spspool,
            tc.tile_pool(name="ps", bufs=2, space="PSUM") as pspool,
        ):
            # --- warm-up (no DMA dependency): ramp the PE clock -----------
            warm = const.tile([128, 512], BF16)
            nc.vector.memset(warm[:], 0.0)
            wp = spspool.tile([128, HW], F32, tag="sp")
            for k in range(N_WARM):
                nc.tensor.matmul(
                    wp[:, (k % 2) * 512 : (k % 2) * 512 + 512],
                    warm[:, 0:128],
                    warm[:],
                    start=True,
                    stop=True,
                )

            p6 = const.tile([6, HW + 64], BF16)
            nc.sync.dma_start(p6[:], p6_d[:])
            idn64 = const.tile([128, 128], BF16)
            nc.sync.dma_start(idn64[:], ident_d[:])
            # per-partition sign mask for the bitwise |S| (DVE has no abs op)
            amask = const.tile([128, 1], U32)
            nc.vector.memset(amask[:], 0x7FFFFFFF)
            # block-diag pair matrices for the DoubleRow transposes, built by
            # the DMA compute engine: zero once, copy 64*I into the two diag
            # blocks, then multiply by broadcast 1/s1 when it is ready.
            dgall = const.tile([128, 2, 4 * BPC, 256], F8)
            nc.gpsimd.memset(dgall[:], 0.0)

            st = [dict() for _ in range(BPC)]

            def load(b):
                s = st[b]
                s["q"] = qpool.tile([6, HW], BF16, tag="q", name="q")
                nc.sync.dma_start(s["q"][:], q6_d[b])
                s["fa"] = fpool.tile([128, NT, C], BF16, tag="fa", name="fa")
                for tj in range(NT):
                    nc.sync.dma_start(
                        s["fa"][:, tj, :], fsrc_d[b, tj * 128 : (tj + 1) * 128, :]
                    )
                s["ea"] = epool.tile([128, NT, HW], F8, tag="ea", name="ea")
                s["ms"] = stat.tile([128, NT], F32, tag="ms", name="ms")
                s["s1"] = stat.tile([128, NT], F32, tag="s1", name="s1")
                s["r1"] = stat.tile([128, NT], F32, tag="r1", name="r1")
                s["e2"] = e2pool.tile([128, NT, HW], BF16, tag="e2", name="e2")
                s["s2"] = stat.tile([128, NT], F32, tag="s2", name="s2")
                s["r2"] = stat.tile([128, NT], F32, tag="r2", name="r2")

            def s_matmul(b, ti):
                s = st[b]
                sp = spspool.tile([128, HW], F32, tag="sp")
                for nh in range(2):
                    nc.tensor.matmul(
                        sp[:, nh * 512 : (nh + 1) * 512],
                        p6[:, ti * 128 : (ti + 1) * 128],
                        s["q"][:, nh * 512 : (nh + 1) * 512],
                        start=True,
                        stop=True,
                    )
                return sp

            def z_max(b, ti, sp, eng):
                # z = |S5|: DVE path via uint32 sign-mask (fp32 z), ACT path
                # via Abs (fp16 z: z <= ~300 here and entries far below the
                # row max don't matter). Row max (negated) on DVE.
                s = st[b]
                if eng == "dve":
                    zt = zpool.tile([128, HW], F32, tag="z32")
                    nc.vector.tensor_scalar(
                        zt[:].bitcast(U32),
                        sp[:].bitcast(U32),
                        amask[:, 0:1],
                        None,
                        op0=ALU.bitwise_and,
                    )
                else:
                    zt = zpool.tile([128, HW], F16, tag="z16")
                    nc.scalar.activation(zt[:], sp[:], AF.Abs)
                nc.vector.reduce_max(
                    s["ms"][:, ti : ti + 1], zt[:], axis=AX.X, negate=True
                )
                return zt

            def e1_pass(b, ti, zt):
                # e1 = exp(z - m) fp8 with fp32 row-sum accum; on odd ti also
                # r1 = 1/s1 (DVE) and the two diag blocks scaled by r1 (DMA).
                s = st[b]
                nc.scalar.activation(
                    s["ea"][:, ti, :],
                    zt[:],
                    AF.Exp,
                    bias=s["ms"][:, ti : ti + 1],
                    accum_out=s["s1"][:, ti : ti + 1],
                )
                if ti % 2 == 1:
                    g = ti // 2
                    nc.vector.reciprocal(
                        s["r1"][:, ti - 1 : ti + 1], s["s1"][:, ti - 1 : ti + 1]
                    )
                    for m in range(2):
                        nc.vector.tensor_scalar_mul(
                            dgall[:, m, g + 4 * b, m * 128 : (m + 1) * 128],
                            idn64[:],
                            s["r1"][:, ti - 1 + m : ti + m],
                        )

            def t_mms(b, tj):
                # tp[j, i] = 64 * p^T via DoubleRow fp8 matmuls against the
                # block-diag pair matrices.
                s = st[b]
                tp = pspool.tile([128, HW], F32, tag="ps")
                for g in range(4):
                    nc.tensor.matmul(
                        tp[:, g * 256 : (g + 1) * 256],
                        s["ea"][:, 2 * g : 2 * g + 2, tj * 128 : (tj + 1) * 128],
                        dgall[:, :, g + 4 * b, :],
                        start=True,
                        stop=True,
                        perf_mode=PM.DoubleRow,
                    )
                return tp

            def e2_pass(b, tj, tp):
                # e2 = exp(-tp/64) with s2 accum; r2 = 1/s2 (DVE); fold r2
                # into the f rows via DMA multiply (broadcast r2).
                s = st[b]
                nc.scalar.activation(
                    s["e2"][:, tj, :],
                    tp[:],
                    AF.Exp,
                    scale=-1.0 / 64.0,
                    accum_out=s["s2"][:, tj : tj + 1],
                )
                nc.vector.reciprocal(
                    s["r2"][:, tj : tj + 1], s["s2"][:, tj : tj + 1]
                )
                nc.vector.tensor_scalar_mul(
                    s["fa"][:, tj, :], s["fa"][:, tj, :], s["r2"][:, tj : tj + 1]
                )

            def gemm_group(b, g):
                # GEMM: out[i, c] = sum_j e2[j, i] * fw[j, c]; two i-tiles per
                # 2-bank PSUM slot.
                s = st[b]
                og = ogpool.tile([128, 2, C], F32, tag="og")
                for tj in range(NT):
                    for half in range(2):
                        oi = 2 * g + half
                        nc.tensor.matmul(
                            og[:, half, :],
                            s["e2"][:, tj, oi * 128 : (oi + 1) * 128],
                            s["fa"][:, tj, :],
                            start=(tj == 0),
                            stop=(tj == NT - 1),
                        )
                return og

            def evict(b, g, og, eng):
                ob = opool.tile([128, 2, C], BF16)
                if eng == 0:
                    nc.scalar.copy(ob[:], og[:])
                else:
                    nc.vector.tensor_copy(ob[:], og[:])
                nc.sync.dma_start(
                    out_d[b, g * 256 : (g + 1) * 256, :].rearrange(
                        "(t p) c -> p t c", p=128
                    ),
                    ob[:],
                )

            # ---- emission schedule (3-phase software pipeline) ----------
            # A: S(b0) + first softmax of b0     (EW-latency bound)
            # B: S(b1)+z(b1)+e1(b1) interleaved with T(b0)+e2(b0) and the
            #    first two GEMM groups of b0 (k-first paced by the e2 stream)
            # C: remaining GEMM groups of b0 woven with T(b1)+e2(b1), then
            #    GEMM of b1 (PE stays dense and hot throughout)
            load(0)
            load(1)
            for ti in range(NT):
                sp = s_matmul(0, ti)
                zt = z_max(0, ti, sp, eng="dve" if ti % 2 == 0 else "act")
                e1_pass(0, ti, zt)

            def g_step(b, og, g, tj, s):
                for half in range(2):
                    oi = 2 * g + half
                    nc.tensor.matmul(
                        og[:, half, :],
                        s["e2"][:, tj, oi * 128 : (oi + 1) * 128],
                        s["fa"][:, tj, :],
                        start=(tj == 0),
                        stop=(tj == NT - 1),
                    )

            # phase B: S(b1) + first softmax of b1 interleaved with
            # T(b0) + e2(b0) + famul(b0)
            for k in range(NT):
                sp = s_matmul(1, k)
                t_mms_tp = t_mms(0, k)
                e2_pass(0, k, t_mms_tp)
                zt = z_max(1, k, sp, eng="dve" if k % 3 != 1 else "act")
                e1_pass(1, k, zt)

            # phase C: GEMM(b0) groups dense with T(b1)/e2(b1) pairs woven
            # between; b0 evicts go to DVE so ACT stays exclusive to e2(b1)
            # (an ACT-queued evict would delay tp recycling and stall PE).
            b1_todo = list(range(NT))
            for g in range(4):
                og = spspool.tile([128, 2, C], F32, tag="sp")
                for tj in range(NT):
                    g_step(0, og, g, tj, st[0])
                    if tj in (2, 5) and b1_todo:
                        kk = b1_todo.pop(0)
                        tp = t_mms(1, kk)
                        e2_pass(1, kk, tp)
                evict(0, g, og, eng=g % 2)
            for kk in b1_todo:
                tp = t_mms(1, kk)
                e2_pass(1, kk, tp)
            for g in range(4):
                og = spspool.tile([128, 2, C], F32, tag="sp")
                for tj in range(NT):
                    g_step(1, og, g, tj, st[1])
                if g < 3:
                    evict(1, g, og, eng=g % 2)
                else:
                    # split the last eviction across both engines so the
                    # final DMA starts sooner
                    ob = opool.tile([128, 2, C], BF16)
                    nc.scalar.copy(ob[:, 0, :], og[:, 0, :])
                    nc.vector.tensor_copy(ob[:, 1, :], og[:, 1, :])
                    nc.sync.dma_start(
                        out_d[1, g * 256 : g * 256 + 128, :].rearrange(
                            "(t p) c -> p t c", p=128
                        ),
                        ob[:, 0:1, :],
                    )
                    nc.sync.dma_start(
                        out_d[1, g * 256 + 128 : (g + 1) * 256, :].rearrange(
                            "(t p) c -> p t c", p=128
                        ),
                        ob[:, 1:2, :],
                    )
    nc.compile()
    return nc


_NC = None


def _get_nc():
    global _NC
    if _NC is None:
        _NC = _build_nc()
    return _NC


# ---------------------------------------------------------------- execution
def _run(inputs, trace=False):
    f_src = np.asarray(inputs["f_src"], np.float32)
    Q, P = _line_coeffs(inputs["K1"], inputs["K2"], inputs["R"], inputs["t"])
    Q5 = 5.0 * Q

    fsrcT = np.ascontiguousarray(
        f_src.reshape(B, C, HW).transpose(0, 2, 1)
    ).astype(ml_dtypes.bfloat16)
    ident64 = (64.0 * np.eye(128, dtype=np.float32)).astype(ml_dtypes.bfloat16)

    q_hi = Q5.astype(ml_dtypes.bfloat16)
    q_lo = (Q5 - q_hi.astype(np.float32)).astype(ml_dtypes.bfloat16)
    q6 = np.concatenate([q_hi, q_lo], axis=1)  # (B, 6, HW) bf16
    # pixel columns plus the 64 coarse-grid nodes (8x8 cell centers)
    nx = np.arange(8, dtype=np.float32) * 4 + 2.0
    cpx, cpy = np.meshgrid(nx, nx, indexing="ij")
    Pc = np.stack(
        [cpx.reshape(-1), cpy.reshape(-1), np.ones(64, np.float32)], axis=0
    )
    Pe = np.concatenate([P, Pc], axis=1)  # (3, HW+64)
    p6 = np.concatenate([Pe, Pe], axis=0).astype(ml_dtypes.bfloat16)  # exact
    # selection matrices: sel[node, ti, p] = 1 iff node owns row 128*ti+p
    pxi = (np.arange(HW) // 32) // 4
    pyi = (np.arange(HW) % 32) // 4
    node_of = pxi * 8 + pyi
    sel = np.zeros((64, NT, 128), np.float32)
    for i in range(HW):
        sel[node_of[i], i // 128, i % 128] = 1.0
    sel = sel.astype(ml_dtypes.bfloat16)

    in_maps = []
    for core in range(NCORES):
        lo = core * BPC
        hi = lo + BPC
        in_maps.append(
            {
                "p6": p6,
                "q6": np.ascontiguousarray(q6[lo:hi]),
                "fsrc": np.ascontiguousarray(fsrcT[lo:hi]),
                "ident64": ident64,
            }
        )

    nc = _get_nc()
    res = run_bass_kernel_spmd(nc, in_maps, list(range(NCORES)), trace=trace)
    out_flat = np.concatenate(
        [res.results[i]["out"] for i in range(NCORES)], axis=0
    )  # (B, HW, C) bf16
    out = np.ascontiguousarray(out_flat).astype(np.float32).reshape(B, C, H, W)
    return out, res


def kernel(**inputs):
    out, _ = _run(inputs, trace=False)
    return out


# revision 36
# speedup vs baseline: 1.0170x; 1.0013x over previous
"""Epipolar attention kernel for Trainium2 (8 NeuronCores, batch-parallel).

Math notes (derived from the reference):
  - f_tar is dead code: the output only depends on f_src / K1 / K2 / R / t.
  - With x0=0, x1=W the distance field factorizes rank-3:
        d[b,i,j] = |px_i*alpha[b,j] + py_i*beta[b,j] + gamma[b,j]|
    so 5*d = |P^T (5Q)| with P = [px;py;1] (exact in bf16).
  - softmax_j(5*(d-0.1)) == softmax_j(5*d)           (shift invariance)
  - softmax_i(1 - p)     == softmax_i(-p), p in (0,1] so exp(-p) needs no max.

Implementation notes (final):
  - S5 = P^T (5Q) via K=6 matmuls: hi/lo bf16 split of 5Q packed into the
    contraction dim -> fp32-grade S5 in ONE matmul pass per 512 columns.
  - z = |S5|: DVE sign-mask (uint32 bitcast, fp32 z) or ACT Abs (fp16 z),
    split across engines to balance; row max on DVE (negated -> exp bias);
    e1 = exp(z-m) stored fp8 with fp32-accumulated row sums s1.
  - transpose via DoubleRow fp8 matmuls (0.5 cyc/row) against block-diagonal
    pair matrices diag(64/s1_t0, 64/s1_t1) built on DVE from a host 64*I.
  - e2 = exp(-tp/64) bf16 with accumulated column sums s2; f rows scaled by
    1/s2 on DVE; GEMM in bf16; bf16 evictions (host casts fp32).
  - schedule: 3-phase software pipeline.  A: S(b0)+softmax1(b0).
    B: S(b1)+softmax1(b1) interleaved with T(b0)+e2(b0) (both EW engines
    ~100% busy).  C: GEMM(b0) groups with T(b1)/e2(b1) pairs woven between
    (ACT kept exclusive to e2 so tp slots recycle without stalling PE),
    then GEMM(b1) dense.  Warm-up matmuls at t=0 ramp the PE p-state.
"""

import numpy as np
import ml_dtypes

import concourse.bass as bass
import concourse.bacc as bacc
import concourse.tile as tile
import concourse.mybir as mybir
from concourse.bass_utils import run_bass_kernel_spmd

B, C, H, W = 16, 512, 32, 32
HW = H * W          # 1024
NCORES = 8
BPC = B // NCORES   # batches per core
NT = HW // 128      # 128-row tiles per HW dim
F32 = mybir.dt.float32
F16 = mybir.dt.float16
U32 = mybir.dt.uint32
BF16 = mybir.dt.bfloat16
F8 = mybir.dt.float8e4
AF = mybir.ActivationFunctionType
AX = mybir.AxisListType
ALU = mybir.AluOpType
PM = mybir.MatmulPerfMode

N_WARM = 7           # warm-up matmuls at t=0 (PE p-state ramp)


# ---------------------------------------------------------------- host math
def _line_coeffs(K1, K2, R, t):
    """Float32 numpy mirror of the reference's per-batch line geometry.

    Returns Q (B, 3, HW) with rows [alpha, beta, gamma] and P (3, HW) with
    rows [px, py, 1].
    """
    K1 = np.asarray(K1, np.float32)
    K2 = np.asarray(K2, np.float32)
    R = np.asarray(R, np.float32)
    t = np.asarray(t, np.float32)

    z = np.zeros_like(t[:, 0])
    tx, ty, tz = t[:, 0], t[:, 1], t[:, 2]
    skew = np.stack(
        [
            np.stack([z, -tz, ty], axis=-1),
            np.stack([tz, z, -tx], axis=-1),
            np.stack([-ty, tx, z], axis=-1),
        ],
        axis=1,
    )
    E = skew @ R
    U, S, Vt = np.linalg.svd(E)
    S = S * np.array([1.0, 1.0, 0.0], dtype=S.dtype)
    E = U @ (S[:, :, None] * Vt)
    Fm = np.linalg.inv(np.swapaxes(K2, 1, 2)) @ E @ np.linalg.inv(K1)
    Fm = Fm.astype(np.float32)

    ix, iy = np.meshgrid(
        np.arange(H, dtype=np.float32), np.arange(W, dtype=np.float32), indexing="ij"
    )
    px = ix.reshape(-1)
    py = iy.reshape(-1)
    idx = np.stack([px, py, np.ones_like(px)], axis=0)  # (3, HW)

    lines = Fm @ idx[None]  # (B, 3, HW)
    a, b, c = lines[:, 0], lines[:, 1], lines[:, 2]
    x0 = np.zeros_like(a)
    y0 = -c / b
    x1 = np.full_like(a, float(W))
    y1 = -(c + a * float(W)) / b
    dx = x0 - x1
    dy = y0 - y1
    L = np.sqrt(dx * dx + dy * dy)

    alpha = dy / L
    beta = -dx / L
    gamma = (y0 * dx) / L
    Q = np.stack([alpha, beta, gamma], axis=1).astype(np.float32)  # (B, 3, HW)
    P = idx.astype(np.float32)
    return Q, P


# ---------------------------------------------------------------- device IR
def _build_nc():
    nc = bacc.Bacc("TRN2", target_bir_lowering=False, debug=False)

    p6_d = nc.dram_tensor("p6", [6, HW + 64], BF16, kind="ExternalInput")
    q6_d = nc.dram_tensor("q6", [BPC, 6, HW], BF16, kind="ExternalInput")
    fsrc_d = nc.dram_tensor("fsrc", [BPC, HW, C], BF16, kind="ExternalInput")
    ident_d = nc.dram_tensor("ident64", [128, 128], BF16, kind="ExternalInput")
    out_d = nc.dram_tensor("out", [BPC, HW, C], BF16, kind="ExternalOutput")

    with tile.TileContext(nc) as tc:
        with (
            tc.tile_pool(name="const", bufs=1) as const,
            tc.tile_pool(name="q", bufs=2) as qpool,
            tc.tile_pool(name="f", bufs=2) as fpool,
            tc.tile_pool(name="z", bufs=12) as zpool,
            tc.tile_pool(name="e", bufs=2) as epool,
            tc.tile_pool(name="e2", bufs=2) as e2pool,
            tc.tile_pool(name="stat", bufs=2) as stat,
            tc.tile_pool(name="o", bufs=4) as opool,
            tc.tile_pool(name="sps", bufs=2, space="PSUM") as spspool,
            tc.tile_pool(name="ps", bufs=2, space="PSUM") as pspool,
        ):
            # --- warm-up (no DMA dependency): ramp the PE clock -----------
            warm = const.tile([128, 512], BF16)
            nc.vector.memset(warm[:], 0.0)
            wp = spspool.tile([128, HW], F32, tag="sp")
            for k in range(N_WARM):
                nc.tensor.matmul(
                    wp[:, (k % 2) * 512 : (k % 2) * 512 + 512],
                    warm[:, 0:128],
                    warm[:],
                    start=True,
                    stop=True,
                )

            p6 = const.tile([6, HW + 64], BF16)
            nc.sync.dma_start(p6[:], p6_d[:])
            idn64 = const.tile([128, 128], BF16)
            nc.sync.dma_start(idn64[:], ident_d[:])
            # per-partition sign mask for the bitwise |S| (DVE has no abs op)
            amask = const.tile([128, 1], U32)
            nc.vector.memset(amask[:], 0x7FFFFFFF)
            # block-diag pair matrices for the DoubleRow transposes, built by
            # the DMA compute engine: zero once, copy 64*I into the two diag
            # blocks, then multiply by broadcast 1/s1 when it is ready.
            dgall = const.tile([128, 2, 4 * BPC, 256], F8)
            nc.gpsimd.memset(dgall[:], 0.0)

            st = [dict() for _ in range(BPC)]

            def load(b):
                s = st[b]
                s["q"] = qpool.tile([6, HW], BF16, tag="q", name="q")
                nc.sync.dma_start(s["q"][:], q6_d[b])
                s["fa"] = fpool.tile([128, NT, C], BF16, tag="fa", name="fa")
                for tj in range(NT):
                    nc.sync.dma_start(
                        s["fa"][:, tj, :], fsrc_d[b, tj * 128 : (tj + 1) * 128, :]
                    )
                s["ea"] = epool.tile([128, NT, HW], F8, tag="ea", name="ea")
                s["ms"] = stat.tile([128, NT], F32, tag="ms", name="ms")
                s["s1"] = stat.tile([128, NT], F32, tag="s1", name="s1")
                s["r1"] = stat.tile([128, NT], F32, tag="r1", name="r1")
                s["e2"] = e2pool.tile([128, NT, HW], BF16, tag="e2", name="e2")
                s["s2"] = stat.tile([128, NT], F32, tag="s2", name="s2")
                s["r2"] = stat.tile([128, NT], F32, tag="r2", name="r2")

            def s_matmul(b, ti):
                s = st[b]
                sp = spspool.tile([128, HW], F32, tag="sp")
                for nh in range(2):
                    nc.tensor.matmul(
                        sp[:, nh * 512 : (nh + 1) * 512],
                        p6[:, ti * 128 : (ti + 1) * 128],
                        s["q"][:, nh * 512 : (nh + 1) * 512],
                        start=True,
                        stop=True,
                    )
                return sp

            def z_max(b, ti, sp, eng):
                # z = |S5|: DVE path via uint32 sign-mask (fp32 z), ACT path
                # via Abs (fp16 z: z <= ~300 here and entries far below the
                # row max don't matter). Row max (negated) on DVE.
                s = st[b]
                if eng == "dve":
                    zt = zpool.tile([128, HW], F32, tag="z32")
                    nc.vector.tensor_scalar(
                        zt[:].bitcast(U32),
                        sp[:].bitcast(U32),
                        amask[:, 0:1],
                        None,
                        op0=ALU.bitwise_and,
                    )
                else:
                    zt = zpool.tile([128, HW], F16, tag="z16")
                    nc.scalar.activation(zt[:], sp[:], AF.Abs)
                nc.vector.reduce_max(
                    s["ms"][:, ti : ti + 1], zt[:], axis=AX.X, negate=True
                )
                return zt

            def e1_pass(b, ti, zt):
                # e1 = exp(z - m) fp8 with fp32 row-sum accum; on odd ti also
                # r1 = 1/s1 (DVE) and the two diag blocks scaled by r1 (DMA).
                s = st[b]
                nc.scalar.activation(
                    s["ea"][:, ti, :],
                    zt[:],
                    AF.Exp,
                    bias=s["ms"][:, ti : ti + 1],
                    accum_out=s["s1"][:, ti : ti + 1],
                )
                if ti % 2 == 1:
                    g = ti // 2
                    nc.vector.reciprocal(
                        s["r1"][:, ti - 1 : ti + 1], s["s1"][:, ti - 1 : ti + 1]
                    )
                    for m in range(2):
                        nc.vector.tensor_scalar_mul(
                            dgall[:, m, g + 4 * b, m * 128 : (m + 1) * 128],
                            idn64[:],
                            s["r1"][:, ti - 1 + m : ti + m],
                        )

            def t_mms(b, tj):
                # tp[j, i] = 64 * p^T via DoubleRow fp8 matmuls against the
                # block-diag pair matrices.
                s = st[b]
                tp = pspool.tile([128, HW], F32, tag="ps")
                for g in range(4):
                    nc.tensor.matmul(
                        tp[:, g * 256 : (g + 1) * 256],
                        s["ea"][:, 2 * g : 2 * g + 2, tj * 128 : (tj + 1) * 128],
                        dgall[:, :, g + 4 * b, :],
                        start=True,
                        stop=True,
                        perf_mode=PM.DoubleRow,
                    )
                return tp

            def e2_pass(b, tj, tp):
                # e2 = exp(-tp/64) with s2 accum; r2 = 1/s2 (DVE); fold r2
                # into the f rows via DMA multiply (broadcast r2).
                s = st[b]
                nc.scalar.activation(
                    s["e2"][:, tj, :],
                    tp[:],
                    AF.Exp,
                    scale=-1.0 / 64.0,
                    accum_out=s["s2"][:, tj : tj + 1],
                )
                nc.vector.reciprocal(
                    s["r2"][:, tj : tj + 1], s["s2"][:, tj : tj + 1]
                )
                nc.vector.tensor_scalar_mul(
                    s["fa"][:, tj, :], s["fa"][:, tj, :], s["r2"][:, tj : tj + 1]
                )

            def gemm_group(b, g):
                # GEMM: out[i, c] = sum_j e2[j, i] * fw[j, c]; two i-tiles per
                # 2-bank PSUM slot.
                s = st[b]
                og = ogpool.tile([128, 2, C], F32, tag="og")
                for tj in range(NT):
                    for half in range(2):
                        oi = 2 * g + half
                        nc.tensor.matmul(
                            og[:, half, :],
                            s["e2"][:, tj, oi * 128 : (oi + 1) * 128],
                            s["fa"][:, tj, :],
                            start=(tj == 0),
                            stop=(tj == NT - 1),
                        )
                return og

            def evict(b, g, og, eng):
                ob = opool.tile([128, 2, C], BF16)
                if eng == 0:
                    nc.scalar.copy(ob[:], og[:])
                else:
                    nc.vector.tensor_copy(ob[:], og[:])
                nc.sync.dma_start(
                    out_d[b, g * 256 : (g + 1) * 256, :].rearrange(
                        "(t p) c -> p t c", p=128
                    ),
                    ob[:],
                )

            # ---- emission schedule (3-phase software pipeline) ----------
            # A: S(b0) + first softmax of b0     (EW-latency bound)
            # B: S(b1)+z(b1)+e1(b1) interleaved with T(b0)+e2(b0) and the
            #    first two GEMM groups of b0 (k-first paced by the e2 stream)
            # C: remaining GEMM groups of b0 woven with T(b1)+e2(b1), then
            #    GEMM of b1 (PE stays dense and hot throughout)
            load(0)
            load(1)
            for ti in range(NT):
                sp = s_matmul(0, ti)
                zt = z_max(0, ti, sp, eng="dve" if ti % 2 == 0 else "act")
                e1_pass(0, ti, zt)

            def g_step(b, og, g, tj, s):
                for half in range(2):
                    oi = 2 * g + half
                    nc.tensor.matmul(
                        og[:, half, :],
                        s["e2"][:, tj, oi * 128 : (oi + 1) * 128],
                        s["fa"][:, tj, :],
                        start=(tj == 0),
                        stop=(tj == NT - 1),
                    )

            # phase B: S(b1) + first softmax of b1 interleaved with
            # T(b0) + e2(b0) + famul(b0)
            for k in range(NT):
                sp = s_matmul(1, k)
                t_mms_tp = t_mms(0, k)
                e2_pass(0, k, t_mms_tp)
                zt = z_max(1, k, sp, eng="dve" if k % 3 != 1 else "act")
                e1_pass(1, k, zt)

            # phase C: GEMM(b0) groups dense with T(b1)/e2(b1) pairs woven
            # between; b0 evicts go to DVE so ACT stays exclusive to e2(b1)
            # (an ACT-queued evict would delay tp recycling and stall PE).
            b1_todo = list(range(NT))
            for g in range(4):
                og = spspool.tile([128, 2, C], F32, tag="sp")
                for tj in range(NT):
                    g_step(0, og, g, tj, st[0])
                    if tj in (2, 5) and b1_todo:
                        kk = b1_todo.pop(0)
                        tp = t_mms(1, kk)
                        e2_pass(1, kk, tp)
                evict(0, g, og, eng=g % 2)
            for kk in b1_todo:
                tp = t_mms(1, kk)
                e2_pass(1, kk, tp)
            for g in range(4):
                og = spspool.tile([128, 2, C], F32, tag="sp")
                for tj in range(NT):
                    g_step(1, og, g, tj, st[1])
                if g < 3:
                    evict(1, g, og, eng=g % 2)
                else:
                    # split the last eviction across both engines so the
                    # final DMA starts sooner
                    ob = opool.tile([128, 2, C], BF16)
                    nc.scalar.copy(ob[:, 0, :], og[:, 0, :])
                    nc.vector.tensor_copy(ob[:, 1, :], og[:, 1, :])
                    nc.sync.dma_start(
                        out_d[1, g * 256 : g * 256 + 128, :].rearrange(
                            "(t p) c -> p t c", p=128
                        ),
                        ob[:, 0:1, :],
                    )
                    nc.sync.dma_start(
                        out_d[1, g * 256 + 128 : (g + 1) * 256, :].rearrange(
                            "(t p) c -> p t c", p=128
                        ),
                        ob[:, 1:2, :],
                    )
    nc.compile()
    return nc


_NC = None


def _get_nc():
    global _NC
    if _NC is None:
        _NC = _build_nc()
    return _NC


# ---------------------------------------------------------------- execution
def _run(inputs, trace=False):
    f_src = np.asarray(inputs["f_src"], np.float32)
    Q, P = _line_coeffs(inputs["K1"], inputs["K2"], inputs["R"], inputs["t"])
    Q5 = 5.0 * Q

    fsrcT = np.ascontiguousarray(
        f_src.reshape(B, C, HW).transpose(0, 2, 1)
    ).astype(ml_dtypes.bfloat16)
    ident64 = (64.0 * np.eye(128, dtype=np.float32)).astype(ml_dtypes.bfloat16)

    q_hi = Q5.astype(ml_dtypes.bfloat16)
    q_lo = (Q5 - q_hi.astype(np.float32)).astype(ml_dtypes.bfloat16)
    q6 = np.concatenate([q_hi, q_lo], axis=1)  # (B, 6, HW) bf16
    # pixel columns plus the 64 coarse-grid nodes (8x8 cell centers)
    nx = np.arange(8, dtype=np.float32) * 4 + 2.0
    cpx, cpy = np.meshgrid(nx, nx, indexing="ij")
    Pc = np.stack(
        [cpx.reshape(-1), cpy.reshape(-1), np.ones(64, np.float32)], axis=0
    )
    Pe = np.concatenate([P, Pc], axis=1)  # (3, HW+64)
    p6 = np.concatenate([Pe, Pe], axis=0).astype(ml_dtypes.bfloat16)  # exact
    # selection matrices: sel[node, ti, p] = 1 iff node owns row 128*ti+p
    pxi = (np.arange(HW) // 32) // 4
    pyi = (np.arange(HW) % 32) // 4
    node_of = pxi * 8 + pyi
    sel = np.zeros((64, NT, 128), np.float32)
    for i in range(HW):
        sel[node_of[i], i // 128, i % 128] = 1.0
    sel = sel.astype(ml_dtypes.bfloat16)

    in_maps = []
    for core in range(NCORES):
        lo = core * BPC
        hi = lo + BPC
        in_maps.append(
            {
                "p6": p6,
                "q6": np.ascontiguousarray(q6[lo:hi]),
                "fsrc": np.ascontiguousarray(fsrcT[lo:hi]),
                "ident64": ident64,
            }
        )

    nc = _get_nc()
    res = run_bass_kernel_spmd(nc, in_maps, list(range(NCORES)), trace=trace)
    out_flat = np.concatenate(
        [res.results[i]["out"] for i in range(NCORES)], axis=0
    )  # (B, HW, C) bf16
    out = np.ascontiguousarray(out_flat).astype(np.float32).reshape(B, C, H, W)
    return out, res


def kernel(**inputs):
    out, _ = _run(inputs, trace=False)
    return out
